# revision 20
# baseline (speedup 1.0000x reference)
"""Batched causal self-attention (B=4, T=2048, C=1024, H=16) on 8 trn2 NeuronCores.

Sharding: data-parallel over B (4) x tensor-parallel over head-halves (2).
Core c handles batch b=c//2, heads [hh*8, hh*8+8) with hh=c%2. Each core
computes its qkv projection slice, causal attention for its 8 heads, and a
partial output projection (512 rows of W_proj); the host sums the two
partials per batch (the TP all-reduce).

Per-core pipeline (bf16 front-end, fp32r output projection):
  phase 1: v = x @ Wv -> v_aug [128, 16, 8*(64+64)] bf16; 64 replicated ones
           columns per head produce the softmax denominator inside the AV
           matmul, replicated across av psum partitions 64:128 so the
           normalization divides without any partition broadcast.
  phase 2 (per head-pair): qT/kT = Wqk-pair.T @ x.T (bf16), then per 512-wide
           query chunk, superslots of two key blocks: S^T [128,1024] = two
           k-block.T @ qT matmuls per head (heads packed into PE row groups
           0-1/2-3, K=64 each); exp on ACT over the 2-bank PSUM tile
           (scale=1/8 folded; no max-subtraction needed, scores ~N(0,0.4^2));
           causal 0/1 mask multiply on the diagonal band; AV psum [128,512]
           accumulation (v_aug as 128-col weight blocks: 64 v-dims, a ones
           column for the softmax denominator, zero padding for FWL) runs 2
           superslots behind S so its wait on exp is already satisfied when
           the PE reaches it; fully-masked leading columns of band blocks are
           skipped in the AV matmuls.
           The attention inner loop is ACT(exp)-bound, and engines run their
           queues in order — so independent PE work (the next pair's qk
           projection, the output projection for finished query chunks) is
           interleaved into the superslot emission via a backlog queue to
           keep the PE busy (and HAM warm) under the exp shadow.
  phase 3: remaining output projection tail (fp32r).
"""

import numpy as np
import ml_dtypes

import concourse.bass as bass
import concourse.mybir as mybir
import concourse.tile as tile
from concourse import bacc
from concourse.bass import ds, ts
from concourse.bass_utils import run_bass_kernel_spmd

B, T, C, H = 4, 2048, 1024, 16
D = 64
NCORES = 8
NPAIR = 4              # head pairs per core (8 heads)
NK = C // 128          # 8 contraction tiles over C
NT = T // 128          # 16 tiles over T
NCH = T // 512         # 4 query chunks
INV_SCALE = 0.125      # 1 / sqrt(C // H)

f32 = mybir.dt.float32
f32r = mybir.dt.float32r
bf16 = mybir.dt.bfloat16
fp16 = mybir.dt.float16

_cache = {}
LAST_RESULTS = None    # test harness reads exec_time_ns from here

# If the caller sets BASS_TRACE=1, run_bass_kernel_spmd imports
# antenv.axon_hooks, which some container images don't ship. Provide a stub
# so tracing degrades gracefully instead of raising ImportError.
try:
    import antenv.axon_hooks  # noqa: F401
except ImportError:
    import sys as _sys
    import types as _types

    _m = _types.ModuleType("antenv.axon_hooks")
    _m._hook = None
    _m.set_axon_ntff_profile_hook = lambda h: setattr(_m, "_hook", h)
    _m.get_axon_ntff_profile_hook = lambda: _m._hook
    _sys.modules["antenv.axon_hooks"] = _m
    # The boot shim registers the NTFF hook only when antenv.axon_hooks is
    # importable at interpreter start — too early for this stub. Register it
    # here instead so BASS_TRACE=1 produces a profile.
    try:
        from trn_agent_boot.trn_boot import _ntff_profile_via_ctypes

        _hk = _ntff_profile_via_ctypes("/opt/axon/libaxon_pjrt.so")
        if _hk is not None:
            _m.set_axon_ntff_profile_hook(_hk)
    except Exception:
        pass


def _build():
    nc = bacc.Bacc("TRN2", target_bir_lowering=False, debug=False)
    xT_d = nc.dram_tensor("xT", [C, T], bf16, kind="ExternalInput").ap()
    wqk_d = nc.dram_tensor("wqk", [8, NK, 128, 128], bf16, kind="ExternalInput").ap()
    wv_d = nc.dram_tensor("wv", [C, 512], bf16, kind="ExternalInput").ap()
    wp_d = nc.dram_tensor("wp", [512, C], fp16, kind="ExternalInput").ap()
    mask_d = nc.dram_tensor("mask", [128, 2, 1024], bf16, kind="ExternalInput").ap()
    out_d = nc.dram_tensor("out", [T, C], f32, kind="ExternalOutput").ap()

    Exp = mybir.ActivationFunctionType.Exp

    with tile.TileContext(nc) as tc:
        with tc.tile_pool(name="persist", bufs=1) as persist:
            # per-head blocks padded to 128 cols (v[0:64] | ones at 64 | zeros)
            # so the AV matmul's weight load is exactly 128 columns -> FWL.
            v_aug = persist.tile([128, NT, 8 * 128], bf16, tag="vaug")
            yT = persist.tile([128, NPAIR, T], fp16, tag="yT")
            mask_t = persist.tile([128, 2, 1024], bf16, tag="mask")
            wp_t = persist.tile([128, 4, C], fp16, tag="wp")

            with (
                tc.tile_pool(name="xpool", bufs=1) as xpool,
                tc.tile_pool(name="wvpool", bufs=1) as wvpool,
                tc.tile_pool(name="wqkpool", bufs=2) as wqkpool,
                tc.tile_pool(name="qkpool", bufs=2) as qkpool,
                tc.tile_pool(name="epool", bufs=8) as epool,
                tc.tile_pool(name="npool", bufs=2) as npool,
                tc.tile_pool(name="opool", bufs=4) as opool,
                tc.tile_pool(name="spsum", bufs=3, space="PSUM") as spsum,
                tc.tile_pool(name="avpsum", bufs=2, space="PSUM") as avpsum,
            ):
                xT_t = xpool.tile([128, NK, T], bf16)

                # ---- phase 1: v projection (streams xT in; xT stays) ----
                # DMA order matches consumption: wv + the h0 halves of each
                # xT k-tile feed pass A (t 0..7); h1 halves feed pass B. k=0
                # h0 is further quartered so the first matmul starts ASAP.
                wv_t = wvpool.tile([128, NK, 512], bf16)
                for k in range(NK):
                    nc.sync.dma_start(wv_t[:, k], wv_d[ds(k * 128, 128)])
                    if k == 0:
                        for q in range(2):
                            nc.sync.dma_start(
                                xT_t[:, 0, ds(q * 512, 512)],
                                xT_d[ds(0, 128), ds(q * 512, 512)])
                    else:
                        nc.sync.dma_start(
                            xT_t[:, k, ds(0, 1024)],
                            xT_d[ds(k * 128, 128), ds(0, 1024)])
                nc.sync.dma_start(mask_t[:], mask_d)
                for k in range(NK):
                    nc.sync.dma_start(
                        xT_t[:, k, ds(1024, 1024)],
                        xT_d[ds(k * 128, 128), ds(1024, 1024)])
                va4 = v_aug.rearrange("p n (h e) -> p n h e", e=128)
                # cols D:128 of each head block are ones: the AV matmul then
                # yields the softmax denominator replicated on partitions
                # D:128 of the av psum — no partition broadcast needed for the
                # normalization. cols 0:D are fully overwritten by the v copy.
                nc.gpsimd.memset(va4[:, :, :, D:128], 1.0)
                # warmup: stream zero matmuls while the first xT tiles are in
                # flight so the HAM clock gate is already at full rate when
                # the real work starts.
                wu = epool.tile([128, 1024], bf16, tag="e", name="warm")
                nc.vector.memset(wu.bitcast(mybir.dt.uint16), 0)
                wups = spsum.tile([128, 1024], f32, tag="s", name="warmps")
                for i in range(14):
                    nc.tensor.matmul(wups[:, ds(512 * (i % 2), 512)],
                                     wu[:, ds(128 * (i % 3), 128)],
                                     wu[:, ds(0, 512)], start=True, stop=True)
                # pass A of the v projection (t 0..7), k-outer so each xT
                # k-tile is consumed for ~1.7us before the next must have
                # landed; 8 concurrent accumulations borrow the idle av pool.
                # Pass B (t 8..15) is deferred into the pair-0 backlog — those
                # v blocks are first needed by attention chunk 2.
                pstiles = [spsum.tile([128, 1024], f32, tag="s",
                                      name=f"vpsA{i}") for i in range(3)]
                avtiles = [avpsum.tile([128, 512], f32, tag="av",
                                       name=f"vpaA{i}") for i in range(2)]
                dsts = [pstiles[i][:, ds(h * 512, 512)]
                        for i in range(3) for h in range(2)]
                dsts += [avtiles[0][:], avtiles[1][:]]
                for k in range(NK):
                    for j in range(8):
                        nc.tensor.matmul(
                            dsts[j], xT_t[:, k, ts(j, 128)], wv_t[:, k],
                            start=(k == 0), stop=(k == NK - 1),
                        )
                for i in range(3):
                    nc.vector.tensor_copy(
                        va4[:, 2 * i:2 * i + 2, :, 0:D],
                        pstiles[i].rearrange("p (n h d) -> p n h d",
                                             n=2, d=D))
                for j in (6, 7):
                    nc.vector.tensor_copy(
                        va4[:, j, :, 0:D],
                        avtiles[j - 6].rearrange("p (h d) -> p h d", d=D))
                for kp in range(4):
                    nc.sync.dma_start(wp_t[:, kp], wp_d[ds(kp * 128, 128)])

                def vpassB_piece(i):
                    def go():
                        pt = spsum.tile([128, 1024], f32, tag="s",
                                        name=f"vpsB{i}")
                        for k in range(NK):
                            for h in range(2):
                                nc.tensor.matmul(
                                    pt[:, ds(h * 512, 512)],
                                    xT_t[:, k, ts(8 + 2 * i + h, 128)],
                                    wv_t[:, k],
                                    start=(k == 0), stop=(k == NK - 1))
                        nc.vector.tensor_copy(
                            va4[:, 8 + 2 * i:10 + 2 * i, :, 0:D],
                            pt.rearrange("p (n h d) -> p n h d", n=2, d=D))
                    return go

                # ---- helpers for interleavable PE work units ----
                qk_tiles = {}

                def start_pair_w(p):
                    def go():
                        wpair = wqkpool.tile([128, NK, 256], bf16, tag="w",
                                             name=f"wpair{p}")
                        for m2 in range(2):
                            nc.sync.dma_start(
                                wpair[:, :, ds(m2 * 128, 128)],
                                wqk_d[4 * m2 + p].rearrange("ko p m -> p ko m"))
                        qk_tiles[p] = (
                            qkpool.tile([128, 2, T], bf16, tag="qk", name=f"qk{p}"),
                            wpair,
                        )
                    return go

                def qkproj_group(p, m2, n):
                    def go():
                        qk, wpair = qk_tiles[p]
                        ps = spsum.tile([128, 512], f32, tag="s",
                                        name=f"qkps{p}_{m2}_{n}")
                        for k in range(NK):
                            nc.tensor.matmul(
                                ps[:], wpair[:, k, ds(m2 * 128, 128)],
                                xT_t[:, k, ds(n * 512, 512)],
                                start=(k == 0), stop=(k == NK - 1))
                        nc.vector.tensor_copy(qk[:, m2, ds(n * 512, 512)], ps[:])
                    return go

                o_tiles = {}

                def proj_group(t, n2):
                    def go():
                        o_t = opool.tile([128, 512], f32, tag="o",
                                         name=f"o{t}_{n2}")
                        ps = spsum.tile([128, 512], f32, tag="s",
                                        name=f"pps{t}_{n2}")
                        for kp in range(4):
                            nc.tensor.matmul(
                                ps[:], yT[:, kp, ts(t, 128)],
                                wp_t[:, kp, ds(n2 * 512, 512)],
                                start=(kp == 0), stop=(kp == 3))
                        nc.vector.tensor_copy(o_t[:], ps[:])
                        nc.sync.dma_start(
                            out_d[ds(t * 128, 128), ds(n2 * 512, 512)], o_t[:])
                    return go

                # pair 0's projection runs up front (nothing to hide it under)
                start_pair_w(0)()
                for n in range(NCH):
                    for m2 in range(2):
                        qkproj_group(0, m2, n)()

                deferred_norm = []

                def make_norm(av, head, p, c):
                    def go():
                        # av rows D:128 hold the denominator replicated by the
                        # ones columns of v_aug — the replication IS the
                        # partition broadcast. One cross-base copy brings it
                        # to partitions 0:D (engines need matching input
                        # bases), then reciprocal + multiply, all from PSUM.
                        rb = npool.tile([D, 512], f32, tag="rb",
                                        name=f"rb{p}_{c}_{head}")
                        nc.vector.tensor_copy(rb[:], av[D:128, :])
                        rr = npool.tile([D, 512], f32, tag="rr",
                                        name=f"rr{p}_{c}_{head}")
                        nc.vector.reciprocal_approx_fast(out=rr[:], in_=rb[:])
                        nc.vector.tensor_mul(
                            yT[ds(D * head, D), p, ds(c * 512, 512)],
                            av[0:D, :], rr[:])
                    return go

                # ---- phase 2: attention per pair, with PE backlog interleave ----
                # `carry` holds backlog items deliberately left over at each
                # pair boundary: they fill the next pair's chunk-0 superslots
                # (which otherwise have no spare PE work and leave the PE
                # idle long enough for the HAM clock gate to drop it to 1/2
                # clock). Items are n-major so carried items are only needed
                # by late chunks of the next pair.
                carry = []
                for p in range(NPAIR):
                    backlog = list(carry)
                    carry = []
                    if p == 0:
                        # deferred v-projection pass B: pure shadow work for
                        # pair 0's exp-bound superslots. Piece i covers v
                        # blocks 8+2i..9+2i, first needed by chunk (4+2i)//4.
                        backlog = [vpassB_piece(i) for i in range(4)] + backlog
                    if p + 1 < NPAIR:
                        backlog.append(start_pair_w(p + 1))
                        for n in range(NCH):
                            for m2 in range(2):
                                backlog.append(qkproj_group(p + 1, m2, n))
                    qk, _ = qk_tiles[p]
                    total_slots = sum(2 * (cc + 1) + 2 for cc in range(NCH))
                    hold = 3 if p + 1 < NPAIR else 3
                    done_slots = 0
                    emitted = 0
                    for c in range(NCH):
                        if p == 0 and c >= 2:
                            # checkpoint: v blocks for this chunk's AV must be
                            # emitted before the chunk's superslots run
                            need = 2 if c == 2 else 4
                            while emitted < need:
                                backlog[emitted]()
                                emitted += 1
                        if p == NPAIR - 1 and c >= 1:
                            # projection for query chunk c-1 is complete
                            for t in range(4 * (c - 1), 4 * c):
                                for n2 in range(2):
                                    backlog.append(proj_group(t, n2))
                        nblk = 4 * (c + 1)
                        nsuper = nblk // 2
                        av_A = avpsum.tile([128, 512], f32, tag="av",
                                           name=f"avA{p}_{c}")
                        av_B = avpsum.tile([128, 512], f32, tag="av",
                                           name=f"avB{p}_{c}")
                        pend = {}
                        drain_at = 0
                        for s in range(nsuper + 2):
                            if s == drain_at and deferred_norm:
                                for fn in deferred_norm:
                                    fn()
                                deferred_norm.clear()
                            if s < nsuper:
                                sA = spsum.tile([128, 1024], f32, tag="s",
                                                name=f"sA{p}_{c}_{s}")
                                sB = spsum.tile([128, 1024], f32, tag="s",
                                                name=f"sB{p}_{c}_{s}")
                                i = s - (nsuper - 2)
                                if i < 0:
                                    for half in (0, 1):
                                        tj = 2 * s + half
                                        nc.tensor.matmul(
                                            sA[:, ds(half * 512, 512)],
                                            qk[0:D, 1, ts(tj, 128)],
                                            qk[0:D, 0, ds(c * 512, 512)],
                                            start=True, stop=True)
                                        nc.tensor.matmul(
                                            sB[:, ds(half * 512, 512)],
                                            qk[D:128, 1, ts(tj, 128)],
                                            qk[D:128, 0, ds(c * 512, 512)],
                                            start=True, stop=True)
                                else:
                                    # band: columns [0, 128d) are fully masked
                                    # — compute only the live range
                                    for half in (0, 1):
                                        tj = 2 * s + half
                                        off = 128 * (2 * i + half)
                                        w = 512 - off
                                        nc.tensor.matmul(
                                            sA[:, ds(half * 512 + off, w)],
                                            qk[0:D, 1, ts(tj, 128)],
                                            qk[0:D, 0, ds(c * 512 + off, w)],
                                            start=True, stop=True)
                                        nc.tensor.matmul(
                                            sB[:, ds(half * 512 + off, w)],
                                            qk[D:128, 1, ts(tj, 128)],
                                            qk[D:128, 0, ds(c * 512 + off, w)],
                                            start=True, stop=True)
                                e_A = epool.tile([128, 1024], bf16, tag="e",
                                                 name=f"eA{p}_{c}_{s}")
                                e_B = epool.tile([128, 1024], bf16, tag="e",
                                                 name=f"eB{p}_{c}_{s}")
                                if i >= 0:
                                    for half in (0, 1):
                                        off = 128 * (2 * i + half)
                                        w = 512 - off
                                        sl = ds(half * 512 + off, w)
                                        nc.scalar.activation(e_A[:, sl], sA[:, sl], Exp,
                                                             scale=INV_SCALE)
                                        nc.scalar.activation(e_B[:, sl], sB[:, sl], Exp,
                                                             scale=INV_SCALE)
                                        # only the leading 128 cols of the
                                        # live range touch the causal
                                        # diagonal; beyond it the mask is 1
                                        dg = ds(half * 512 + off, 128)
                                        nc.vector.tensor_mul(e_A[:, dg], e_A[:, dg],
                                                             mask_t[:, 0, 0:128])
                                        nc.vector.tensor_mul(e_B[:, dg], e_B[:, dg],
                                                             mask_t[:, 0, 0:128])
                                else:
                                    nc.scalar.activation(e_A[:], sA[:], Exp,
                                                         scale=INV_SCALE)
                                    nc.scalar.activation(e_B[:], sB[:], Exp,
                                                         scale=INV_SCALE)
                                pend[s] = (e_A, e_B)
                            if s >= 2:
                                e_A, e_B = pend.pop(s - 2)
                                s2 = s - 2
                                for half in (0, 1):
                                    tj = 2 * s2 + half
                                    dd = tj - (nblk - 4)
                                    # masked band columns [0,128d) of E are
                                    # zero after the mask multiply — skip them
                                    off = 128 * dd if dd > 0 else 0
                                    w = 512 - off
                                    nc.tensor.matmul(
                                        av_A[:, ds(off, w)],
                                        v_aug[:, tj, ds(2 * p * 128, 128)],
                                        e_A[:, ds(half * 512 + off, w)],
                                        start=(tj == 0), stop=(tj == nblk - 1))
                                    nc.tensor.matmul(
                                        av_B[:, ds(off, w)],
                                        v_aug[:, tj, ds((2 * p + 1) * 128, 128)],
                                        e_B[:, ds(half * 512 + off, w)],
                                        start=(tj == 0), stop=(tj == nblk - 1))
                            # pace the backlog across the pair's superslots
                            # (with a 2-slot head start so the early, exp-
                            # bound chunks stay fed), but hold back `hold`
                            # items for the next pair's chunk 0 / the tail.
                            done_slots += 1
                            if p == NPAIR - 1:
                                limit = len(backlog) - (hold if c == NCH - 1 else 0)
                                if emitted < limit:
                                    backlog[emitted]()
                                    emitted += 1
                            else:
                                avail = len(backlog) - hold
                                want = -(-avail * min(done_slots + 3, total_slots)
                                         // total_slots)
                                while emitted < min(want, avail):
                                    backlog[emitted]()
                                    emitted += 1
                        for head, av in ((0, av_A), (1, av_B)):
                            deferred_norm.append(make_norm(av, head, p, c))
                    carry = backlog[emitted:]

                # reserved proj groups bridge the PE across the final norms
                # (which the tail projection depends on), keeping HAM warm.
                for fn in carry:
                    fn()
                for fn in deferred_norm:
                    fn()
                deferred_norm.clear()

                # ---- phase 3: projection tail (last query chunk) ----
                for t in range(12, NT):
                    for n2 in range(2):
                        proj_group(t, n2)()

    nc.compile()
    return nc


def _make_mask():
    # mask[p, i, 512*h2 + j] = 1 iff j >= 128*(2i+h2) + p  (d = 2i + h2)
    p = np.arange(128)[:, None, None]
    jj = np.arange(1024)[None, None, :]
    i = np.arange(2)[None, :, None]
    d = 2 * i + (jj // 512)
    j = jj % 512
    return (j >= 128 * d + p).astype(ml_dtypes.bfloat16)


def kernel(x: np.ndarray, W_attn: np.ndarray, W_proj: np.ndarray) -> np.ndarray:
    global LAST_RESULTS
    x = np.asarray(x, dtype=np.float32)
    W_attn = np.asarray(W_attn, dtype=np.float32)
    W_proj = np.asarray(W_proj, dtype=np.float32)

    nc = _cache.get("nc")
    if nc is None:
        nc = _build()
        _cache["nc"] = nc

    mask = _make_mask()
    xTs = [np.ascontiguousarray(x[b].T).astype(ml_dtypes.bfloat16) for b in range(B)]
    in_maps = []
    for c in range(NCORES):
        b, hh = c // 2, c % 2
        qcols = W_attn[:, hh * 512:(hh + 1) * 512]
        kcols = W_attn[:, C + hh * 512:C + (hh + 1) * 512]
        wqk = np.concatenate([qcols, kcols], axis=1)                  # [1024, 1024]
        wqk_blocks = np.ascontiguousarray(
            wqk.reshape(NK, 128, 8, 128).transpose(2, 0, 1, 3)
        ).astype(ml_dtypes.bfloat16)                                  # [m, ko, p, mm]
        wv = np.ascontiguousarray(
            W_attn[:, 2 * C + hh * 512:2 * C + (hh + 1) * 512]
        ).astype(ml_dtypes.bfloat16)
        wp = np.ascontiguousarray(W_proj[hh * 512:(hh + 1) * 512, :]).astype(np.float16)
        in_maps.append({
            "xT": xTs[b], "wqk": wqk_blocks, "wv": wv, "wp": wp, "mask": mask,
        })

    res = run_bass_kernel_spmd(nc, in_maps, core_ids=list(range(NCORES)))
    LAST_RESULTS = res
    parts = [res.results[c]["out"] for c in range(NCORES)]
    out = np.stack([parts[2 * b] + parts[2 * b + 1] for b in range(B)], axis=0)
    return np.ascontiguousarray(out, dtype=np.float32)



# revision 24
# speedup vs baseline: 1.0344x; 1.0344x over previous
"""Batched causal self-attention (B=4, T=2048, C=1024, H=16) on 8 trn2 NeuronCores.

Sharding: data-parallel over B (4) x tensor-parallel over head-halves (2).
Core c handles batch b=c//2, heads [hh*8, hh*8+8) with hh=c%2. Each core
computes its qkv projection slice, causal attention for its 8 heads, and a
partial output projection (512 rows of W_proj); the host sums the two
partials per batch (the TP all-reduce).

Per-core pipeline (bf16 front-end, fp32r output projection):
  phase 1: v = x @ Wv -> v_aug [128, 16, 8*(64+64)] bf16; 64 replicated ones
           columns per head produce the softmax denominator inside the AV
           matmul, replicated across av psum partitions 64:128 so the
           normalization divides without any partition broadcast.
  phase 2 (per head-pair): qT/kT = Wqk-pair.T @ x.T (bf16), then per 512-wide
           query chunk, superslots of two key blocks: S^T [128,1024] = two
           k-block.T @ qT matmuls per head (heads packed into PE row groups
           0-1/2-3, K=64 each); exp on ACT over the 2-bank PSUM tile
           (scale=1/8 folded; no max-subtraction needed, scores ~N(0,0.4^2));
           causal 0/1 mask multiply on the diagonal band; AV psum [128,512]
           accumulation (v_aug as 128-col weight blocks: 64 v-dims, a ones
           column for the softmax denominator, zero padding for FWL) runs 2
           superslots behind S so its wait on exp is already satisfied when
           the PE reaches it; fully-masked leading columns of band blocks are
           skipped in the AV matmuls.
           The attention inner loop is ACT(exp)-bound, and engines run their
           queues in order — so independent PE work (the next pair's qk
           projection, the output projection for finished query chunks) is
           interleaved into the superslot emission via a backlog queue to
           keep the PE busy (and HAM warm) under the exp shadow.
  phase 3: remaining output projection tail (fp32r).
"""

import numpy as np
import ml_dtypes

import concourse.bass as bass
import concourse.mybir as mybir
import concourse.tile as tile
from concourse import bacc
from concourse.bass import ds, ts
from concourse.bass_utils import run_bass_kernel_spmd

B, T, C, H = 4, 2048, 1024, 16
D = 64
NCORES = 8
NPAIR = 4              # head pairs per core (8 heads)
NK = C // 128          # 8 contraction tiles over C
NT = T // 128          # 16 tiles over T
NCH = T // 512         # 4 query chunks
INV_SCALE = 0.125      # 1 / sqrt(C // H)

f32 = mybir.dt.float32
f32r = mybir.dt.float32r
bf16 = mybir.dt.bfloat16
fp16 = mybir.dt.float16

_cache = {}
LAST_RESULTS = None    # test harness reads exec_time_ns from here

# If the caller sets BASS_TRACE=1, run_bass_kernel_spmd imports
# antenv.axon_hooks, which some container images don't ship. Provide a stub
# so tracing degrades gracefully instead of raising ImportError.
try:
    import antenv.axon_hooks  # noqa: F401
except ImportError:
    import sys as _sys
    import types as _types

    _m = _types.ModuleType("antenv.axon_hooks")
    _m._hook = None
    _m.set_axon_ntff_profile_hook = lambda h: setattr(_m, "_hook", h)
    _m.get_axon_ntff_profile_hook = lambda: _m._hook
    _sys.modules["antenv.axon_hooks"] = _m
    # The boot shim registers the NTFF hook only when antenv.axon_hooks is
    # importable at interpreter start — too early for this stub. Register it
    # here instead so BASS_TRACE=1 produces a profile.
    try:
        from trn_agent_boot.trn_boot import _ntff_profile_via_ctypes

        _hk = _ntff_profile_via_ctypes("/opt/axon/libaxon_pjrt.so")
        if _hk is not None:
            _m.set_axon_ntff_profile_hook(_hk)
    except Exception:
        pass


def _build():
    nc = bacc.Bacc("TRN2", target_bir_lowering=False, debug=False)
    xT_d = nc.dram_tensor("xT", [C, T], bf16, kind="ExternalInput").ap()
    wqk_d = nc.dram_tensor("wqk", [8, NK, 128, 128], bf16, kind="ExternalInput").ap()
    wv_d = nc.dram_tensor("wv", [C, 512], bf16, kind="ExternalInput").ap()
    wp_d = nc.dram_tensor("wp", [512, C], fp16, kind="ExternalInput").ap()
    mask_d = nc.dram_tensor("mask", [128, 2, 1024], bf16, kind="ExternalInput").ap()
    out_d = nc.dram_tensor("out", [T, C], f32, kind="ExternalOutput").ap()

    Exp = mybir.ActivationFunctionType.Exp

    with tile.TileContext(nc) as tc:
        with tc.tile_pool(name="persist", bufs=1) as persist:
            # per-head blocks padded to 128 cols (v[0:64] | ones at 64 | zeros)
            # so the AV matmul's weight load is exactly 128 columns -> FWL.
            v_aug = persist.tile([128, NT, 8 * 128], bf16, tag="vaug")
            yT = persist.tile([128, NPAIR, T], fp16, tag="yT")
            mask_t = persist.tile([128, 2, 1024], bf16, tag="mask")
            wp_t = persist.tile([128, 4, C], fp16, tag="wp")

            with (
                tc.tile_pool(name="xpool", bufs=1) as xpool,
                tc.tile_pool(name="wvpool", bufs=1) as wvpool,
                tc.tile_pool(name="wqkpool", bufs=2) as wqkpool,
                tc.tile_pool(name="qkpool", bufs=2) as qkpool,
                tc.tile_pool(name="epool", bufs=8) as epool,
                tc.tile_pool(name="npool", bufs=2) as npool,
                tc.tile_pool(name="opool", bufs=4) as opool,
                tc.tile_pool(name="spsum", bufs=3, space="PSUM") as spsum,
                tc.tile_pool(name="avpsum", bufs=2, space="PSUM") as avpsum,
            ):
                xT_t = xpool.tile([128, NK, T], bf16)

                # ---- phase 1: v projection (streams xT in; xT stays) ----
                # DMA order matches consumption: wv + the h0 halves of each
                # xT k-tile feed pass A (t 0..7); h1 halves feed pass B. k=0
                # h0 is further quartered so the first matmul starts ASAP.
                wv_t = wvpool.tile([128, NK, 512], bf16)
                for k in range(NK):
                    nc.sync.dma_start(wv_t[:, k], wv_d[ds(k * 128, 128)])
                    if k == 0:
                        for q in range(2):
                            nc.sync.dma_start(
                                xT_t[:, 0, ds(q * 512, 512)],
                                xT_d[ds(0, 128), ds(q * 512, 512)])
                    else:
                        nc.sync.dma_start(
                            xT_t[:, k, ds(0, 1024)],
                            xT_d[ds(k * 128, 128), ds(0, 1024)])
                # pair-0 qk weights right behind the h0 halves: qkproj(0)
                # needs them ~24us in, right after vproj pass A.
                wpair0 = wqkpool.tile([128, NK, 256], bf16, tag="w",
                                      name="wpair0")
                for m2 in range(2):
                    nc.sync.dma_start(
                        wpair0[:, :, ds(m2 * 128, 128)],
                        wqk_d[4 * m2].rearrange("ko p m -> p ko m"))
                for k in range(NK):
                    nc.sync.dma_start(
                        xT_t[:, k, ds(1024, 1024)],
                        xT_d[ds(k * 128, 128), ds(1024, 1024)])
                nc.sync.dma_start(mask_t[:], mask_d)
                va4 = v_aug.rearrange("p n (h e) -> p n h e", e=128)
                # cols D:128 of each head block are ones: the AV matmul then
                # yields the softmax denominator replicated on partitions
                # D:128 of the av psum — no partition broadcast needed for the
                # normalization. cols 0:D are fully overwritten by the v copy.
                nc.gpsimd.memset(va4[:, :, :, D:128], 1.0)
                # warmup: stream zero matmuls while the first xT tiles are in
                # flight so the HAM clock gate is already at full rate when
                # the real work starts.
                wu = epool.tile([128, 1024], bf16, tag="e", name="warm")
                nc.vector.memset(wu.bitcast(mybir.dt.uint16), 0)
                wups = spsum.tile([128, 1024], f32, tag="s", name="warmps")
                for i in range(14):
                    nc.tensor.matmul(wups[:, ds(512 * (i % 2), 512)],
                                     wu[:, ds(128 * (i % 3), 128)],
                                     wu[:, ds(0, 512)], start=True, stop=True)
                # pass A of the v projection (t 0..7), k-outer so each xT
                # k-tile is consumed for ~1.7us before the next must have
                # landed; 8 concurrent accumulations borrow the idle av pool.
                # Pass B (t 8..15) is deferred into the pair-0 backlog — those
                # v blocks are first needed by attention chunk 2.
                pstiles = [spsum.tile([128, 1024], f32, tag="s",
                                      name=f"vpsA{i}") for i in range(3)]
                avtiles = [avpsum.tile([128, 512], f32, tag="av",
                                       name=f"vpaA{i}") for i in range(2)]
                dsts = [pstiles[i][:, ds(h * 512, 512)]
                        for i in range(3) for h in range(2)]
                dsts += [avtiles[0][:], avtiles[1][:]]
                for k in range(NK):
                    for j in range(8):
                        nc.tensor.matmul(
                            dsts[j], xT_t[:, k, ts(j, 128)], wv_t[:, k],
                            start=(k == 0), stop=(k == NK - 1),
                        )
                for i in range(3):
                    nc.vector.tensor_copy(
                        va4[:, 2 * i:2 * i + 2, :, 0:D],
                        pstiles[i].rearrange("p (n h d) -> p n h d",
                                             n=2, d=D))
                for j in (6, 7):
                    nc.vector.tensor_copy(
                        va4[:, j, :, 0:D],
                        avtiles[j - 6].rearrange("p (h d) -> p h d", d=D))
                for kp in range(4):
                    nc.sync.dma_start(wp_t[:, kp], wp_d[ds(kp * 128, 128)])

                def vpassB_piece(i):
                    def go():
                        pt = spsum.tile([128, 1024], f32, tag="s",
                                        name=f"vpsB{i}")
                        for k in range(NK):
                            for h in range(2):
                                nc.tensor.matmul(
                                    pt[:, ds(h * 512, 512)],
                                    xT_t[:, k, ts(8 + 2 * i + h, 128)],
                                    wv_t[:, k],
                                    start=(k == 0), stop=(k == NK - 1))
                        nc.vector.tensor_copy(
                            va4[:, 8 + 2 * i:10 + 2 * i, :, 0:D],
                            pt.rearrange("p (n h d) -> p n h d", n=2, d=D))
                    return go

                # ---- helpers for interleavable PE work units ----
                qk_tiles = {}

                def start_pair_w(p):
                    def go():
                        wpair = wqkpool.tile([128, NK, 256], bf16, tag="w",
                                             name=f"wpair{p}")
                        for m2 in range(2):
                            nc.sync.dma_start(
                                wpair[:, :, ds(m2 * 128, 128)],
                                wqk_d[4 * m2 + p].rearrange("ko p m -> p ko m"))
                        qk_tiles[p] = (
                            qkpool.tile([128, 2, T], bf16, tag="qk", name=f"qk{p}"),
                            wpair,
                        )
                    return go

                def qkproj_group(p, m2, n):
                    def go():
                        qk, wpair = qk_tiles[p]
                        ps = spsum.tile([128, 512], f32, tag="s",
                                        name=f"qkps{p}_{m2}_{n}")
                        for k in range(NK):
                            nc.tensor.matmul(
                                ps[:], wpair[:, k, ds(m2 * 128, 128)],
                                xT_t[:, k, ds(n * 512, 512)],
                                start=(k == 0), stop=(k == NK - 1))
                        nc.vector.tensor_copy(qk[:, m2, ds(n * 512, 512)], ps[:])
                    return go

                o_tiles = {}

                def proj_group(t, n2):
                    def go():
                        o_t = opool.tile([128, 512], f32, tag="o",
                                         name=f"o{t}_{n2}")
                        ps = spsum.tile([128, 512], f32, tag="s",
                                        name=f"pps{t}_{n2}")
                        for kp in range(4):
                            nc.tensor.matmul(
                                ps[:], yT[:, kp, ts(t, 128)],
                                wp_t[:, kp, ds(n2 * 512, 512)],
                                start=(kp == 0), stop=(kp == 3))
                        nc.vector.tensor_copy(o_t[:], ps[:])
                        nc.sync.dma_start(
                            out_d[ds(t * 128, 128), ds(n2 * 512, 512)], o_t[:])
                    return go

                # pair 0's projection runs up front (nothing to hide it
                # under); its weights were DMA'd during phase 1.
                qk_tiles[0] = (
                    qkpool.tile([128, 2, T], bf16, tag="qk", name="qk0"),
                    wpair0,
                )
                for n in range(NCH):
                    for m2 in range(2):
                        qkproj_group(0, m2, n)()

                deferred_norm = []

                def make_norm(av, head, p, c):
                    def go():
                        # av rows D:128 hold the denominator replicated by the
                        # ones columns of v_aug — the replication IS the
                        # partition broadcast. One cross-base copy brings it
                        # to partitions 0:D (engines need matching input
                        # bases), then reciprocal + multiply, all from PSUM.
                        rb = npool.tile([D, 512], f32, tag="rb",
                                        name=f"rb{p}_{c}_{head}")
                        nc.vector.tensor_copy(rb[:], av[D:128, :])
                        rr = npool.tile([D, 512], f32, tag="rr",
                                        name=f"rr{p}_{c}_{head}")
                        nc.vector.reciprocal_approx_fast(out=rr[:], in_=rb[:])
                        nc.vector.tensor_mul(
                            yT[ds(D * head, D), p, ds(c * 512, 512)],
                            av[0:D, :], rr[:])
                    return go

                # ---- phase 2: attention per pair, with PE backlog interleave ----
                # `carry` holds backlog items deliberately left over at each
                # pair boundary: they fill the next pair's chunk-0 superslots
                # (which otherwise have no spare PE work and leave the PE
                # idle long enough for the HAM clock gate to drop it to 1/2
                # clock). Items are n-major so carried items are only needed
                # by late chunks of the next pair.
                carry = []
                for p in range(NPAIR):
                    backlog = list(carry)
                    carry = []
                    if p == 0:
                        # deferred v-projection pass B: pure shadow work for
                        # pair 0's exp-bound superslots. Piece i covers v
                        # blocks 8+2i..9+2i, first needed by chunk (4+2i)//4.
                        backlog = [vpassB_piece(i) for i in range(4)] + backlog
                    if p + 1 < NPAIR:
                        backlog.append(start_pair_w(p + 1))
                        for n in range(NCH):
                            for m2 in range(2):
                                backlog.append(qkproj_group(p + 1, m2, n))
                    qk, _ = qk_tiles[p]
                    total_slots = sum(2 * (cc + 1) + 2 for cc in range(NCH))
                    hold = 3 if p + 1 < NPAIR else 3
                    done_slots = 0
                    emitted = 0
                    for c in range(NCH):
                        if p == 0 and c >= 2:
                            # checkpoint: v blocks for this chunk's AV must be
                            # emitted before the chunk's superslots run
                            need = 2 if c == 2 else 4
                            while emitted < need:
                                backlog[emitted]()
                                emitted += 1
                        if p == NPAIR - 1 and c >= 1:
                            # projection for query chunk c-1 is complete
                            for t in range(4 * (c - 1), 4 * c):
                                for n2 in range(2):
                                    backlog.append(proj_group(t, n2))
                        nblk = 4 * (c + 1)
                        nsuper = nblk // 2
                        av_A = avpsum.tile([128, 512], f32, tag="av",
                                           name=f"avA{p}_{c}")
                        av_B = avpsum.tile([128, 512], f32, tag="av",
                                           name=f"avB{p}_{c}")
                        pend = {}
                        drain_at = 0
                        for s in range(nsuper + 2):
                            if s == drain_at and deferred_norm:
                                for fn in deferred_norm:
                                    fn()
                                deferred_norm.clear()
                            if s < nsuper:
                                sA = spsum.tile([128, 1024], f32, tag="s",
                                                name=f"sA{p}_{c}_{s}")
                                sB = spsum.tile([128, 1024], f32, tag="s",
                                                name=f"sB{p}_{c}_{s}")
                                i = s - (nsuper - 2)
                                if i < 0:
                                    for half in (0, 1):
                                        tj = 2 * s + half
                                        nc.tensor.matmul(
                                            sA[:, ds(half * 512, 512)],
                                            qk[0:D, 1, ts(tj, 128)],
                                            qk[0:D, 0, ds(c * 512, 512)],
                                            start=True, stop=True)
                                        nc.tensor.matmul(
                                            sB[:, ds(half * 512, 512)],
                                            qk[D:128, 1, ts(tj, 128)],
                                            qk[D:128, 0, ds(c * 512, 512)],
                                            start=True, stop=True)
                                else:
                                    # band: columns [0, 128d) are fully masked
                                    # — compute only the live range
                                    for half in (0, 1):
                                        tj = 2 * s + half
                                        off = 128 * (2 * i + half)
                                        w = 512 - off
                                        nc.tensor.matmul(
                                            sA[:, ds(half * 512 + off, w)],
                                            qk[0:D, 1, ts(tj, 128)],
                                            qk[0:D, 0, ds(c * 512 + off, w)],
                                            start=True, stop=True)
                                        nc.tensor.matmul(
                                            sB[:, ds(half * 512 + off, w)],
                                            qk[D:128, 1, ts(tj, 128)],
                                            qk[D:128, 0, ds(c * 512 + off, w)],
                                            start=True, stop=True)
                                e_A = epool.tile([128, 1024], bf16, tag="e",
                                                 name=f"eA{p}_{c}_{s}")
                                e_B = epool.tile([128, 1024], bf16, tag="e",
                                                 name=f"eB{p}_{c}_{s}")
                                if i >= 0:
                                    for half in (0, 1):
                                        off = 128 * (2 * i + half)
                                        w = 512 - off
                                        sl = ds(half * 512 + off, w)
                                        nc.scalar.activation(e_A[:, sl], sA[:, sl], Exp,
                                                             scale=INV_SCALE)
                                        nc.scalar.activation(e_B[:, sl], sB[:, sl], Exp,
                                                             scale=INV_SCALE)
                                        # only the leading 128 cols of the
                                        # live range touch the causal
                                        # diagonal; beyond it the mask is 1
                                        dg = ds(half * 512 + off, 128)
                                        nc.vector.tensor_mul(e_A[:, dg], e_A[:, dg],
                                                             mask_t[:, 0, 0:128])
                                        nc.vector.tensor_mul(e_B[:, dg], e_B[:, dg],
                                                             mask_t[:, 0, 0:128])
                                else:
                                    nc.scalar.activation(e_A[:], sA[:], Exp,
                                                         scale=INV_SCALE)
                                    nc.scalar.activation(e_B[:], sB[:], Exp,
                                                         scale=INV_SCALE)
                                pend[s] = (e_A, e_B)
                            if s >= 2:
                                e_A, e_B = pend.pop(s - 2)
                                s2 = s - 2
                                for half in (0, 1):
                                    tj = 2 * s2 + half
                                    dd = tj - (nblk - 4)
                                    # masked band columns [0,128d) of E are
                                    # zero after the mask multiply — skip them
                                    off = 128 * dd if dd > 0 else 0
                                    w = 512 - off
                                    nc.tensor.matmul(
                                        av_A[:, ds(off, w)],
                                        v_aug[:, tj, ds(2 * p * 128, 128)],
                                        e_A[:, ds(half * 512 + off, w)],
                                        start=(tj == 0), stop=(tj == nblk - 1))
                                    nc.tensor.matmul(
                                        av_B[:, ds(off, w)],
                                        v_aug[:, tj, ds((2 * p + 1) * 128, 128)],
                                        e_B[:, ds(half * 512 + off, w)],
                                        start=(tj == 0), stop=(tj == nblk - 1))
                            # pace the backlog across the pair's superslots
                            # (with a 2-slot head start so the early, exp-
                            # bound chunks stay fed), but hold back `hold`
                            # items for the next pair's chunk 0 / the tail.
                            done_slots += 1
                            if p == NPAIR - 1:
                                limit = len(backlog) - (hold if c == NCH - 1 else 0)
                                if emitted < limit:
                                    backlog[emitted]()
                                    emitted += 1
                            else:
                                avail = len(backlog) - hold
                                want = -(-avail * min(done_slots + 3, total_slots)
                                         // total_slots)
                                while emitted < min(want, avail):
                                    backlog[emitted]()
                                    emitted += 1
                        for head, av in ((0, av_A), (1, av_B)):
                            deferred_norm.append(make_norm(av, head, p, c))
                    carry = backlog[emitted:]

                # final norms first on DVE, then the reserved proj groups:
                # the PE streams the (norm-independent) reserved groups while
                # DVE runs the norms, and the tail starts as soon as the
                # norms land instead of queueing behind the o copies.
                for fn in deferred_norm:
                    fn()
                deferred_norm.clear()
                for fn in carry:
                    fn()

                # ---- phase 3: projection tail (last query chunk) ----
                for t in range(12, NT):
                    for n2 in range(2):
                        proj_group(t, n2)()

    nc.compile()
    return nc


def _make_mask():
    # mask[p, i, 512*h2 + j] = 1 iff j >= 128*(2i+h2) + p  (d = 2i + h2)
    p = np.arange(128)[:, None, None]
    jj = np.arange(1024)[None, None, :]
    i = np.arange(2)[None, :, None]
    d = 2 * i + (jj // 512)
    j = jj % 512
    return (j >= 128 * d + p).astype(ml_dtypes.bfloat16)


def kernel(x: np.ndarray, W_attn: np.ndarray, W_proj: np.ndarray) -> np.ndarray:
    global LAST_RESULTS
    x = np.asarray(x, dtype=np.float32)
    W_attn = np.asarray(W_attn, dtype=np.float32)
    W_proj = np.asarray(W_proj, dtype=np.float32)

    nc = _cache.get("nc")
    if nc is None:
        nc = _build()
        _cache["nc"] = nc

    mask = _make_mask()
    xTs = [np.ascontiguousarray(x[b].T).astype(ml_dtypes.bfloat16) for b in range(B)]
    in_maps = []
    for c in range(NCORES):
        b, hh = c // 2, c % 2
        qcols = W_attn[:, hh * 512:(hh + 1) * 512]
        kcols = W_attn[:, C + hh * 512:C + (hh + 1) * 512]
        wqk = np.concatenate([qcols, kcols], axis=1)                  # [1024, 1024]
        wqk_blocks = np.ascontiguousarray(
            wqk.reshape(NK, 128, 8, 128).transpose(2, 0, 1, 3)
        ).astype(ml_dtypes.bfloat16)                                  # [m, ko, p, mm]
        wv = np.ascontiguousarray(
            W_attn[:, 2 * C + hh * 512:2 * C + (hh + 1) * 512]
        ).astype(ml_dtypes.bfloat16)
        wp = np.ascontiguousarray(W_proj[hh * 512:(hh + 1) * 512, :]).astype(np.float16)
        in_maps.append({
            "xT": xTs[b], "wqk": wqk_blocks, "wv": wv, "wp": wp, "mask": mask,
        })

    res = run_bass_kernel_spmd(nc, in_maps, core_ids=list(range(NCORES)))
    LAST_RESULTS = res
    parts = [res.results[c]["out"] for c in range(NCORES)]
    out = np.stack([parts[2 * b] + parts[2 * b + 1] for b in range(B)], axis=0)
    return np.ascontiguousarray(out, dtype=np.float32)

